# revision 4
# baseline (speedup 1.0000x reference)
"""Trainium2 Bass kernel for a small Elman RNN over a very long sequence.

Model (matches the torch/jax reference):
    xp_t  = W_ih @ x_t + b_ih + b_hh
    h_t   = tanh(xp_t + W_hh @ h_{t-1}),  h_{-1} = 0
    out_t = W_fc @ h_t + b_fc

Chunked-scan with burn-in (the recurrence is strongly contractive, so a
B-step replay of the preceding timesteps from h=0 converges to the true
state).  All matmul operands are fp16 (1 PE pass/row vs 4 for f32r;
fp16's 11-bit mantissa keeps quantization at the f32r-noise level).

Per-core layout (8 cores, each owns TC = 65536 contiguous steps),
NSTREAM=2 interleaved scan streams so one stream's matmuls overlap the
other stream's tanh on the serial chain:
  - G=11 chunk groups x F=176 chunk columns per stream, L=17 real steps
    per chunk, B=10 burn-in steps, S=B+L scan steps.
  - hblk (fp16, [110, (S+1)*F]): h state, group g at partitions
    10g..10g+9; block u holds the h that step u consumes.
  - xt (fp16, [66, (S+1)*F]): src features, group g at partitions
    5g..5g+4; rows 55+g = per-group validity row (1.0 inside the global
    timestep range) that carries bias+b_fc through the stationary --
    this makes the global first chunk's burn-in exact (h stays 0), no
    host fixup needed.
  - Two accumulating matmuls per step per stream into one PSUM tile
    [121, F]: xp = W_ih x + biases (start), then += W_hh h (stop).
    Stationary cols 110..120 hold W_fc, so the same pair also emits the
    PREVIOUS step's fc outputs (+b_fc via the validity rows) for free.
  - ACT tanh PSUM->SBUF writes block u+1 (rows 0..109, fp16).
  - DVE copies PSUM out rows [11, F] into a t-major out_sb tile so the
    output DMA is contiguous; the host untransposes (c, t) for free.
"""

import numpy as np

T = 524288
IN, HID, OUT = 5, 10, 1
NCORES = 8
TC = T // NCORES

G = 11             # chunk groups (10 partitions of h each)
F = 176            # chunk columns per group (matmul free dim)
NSTREAM = 2
B = 12             # burn-in steps
L = 17             # real steps per chunk
S = B + L          # scan steps (tanh per step); S+1 matmul iterations
CS = G * F         # chunks per stream
SEG = TC // NSTREAM
M = 128            # stationary cols: 110 pre + 11 out (+7 zero pad)
XROWS = 5 * G + G  # 55 src rows + 11 per-group ones rows

_COMPILED = {}


def _build_kernel():
    import concourse.bacc as bacc
    import concourse.mybir as mybir
    from concourse import tile

    dt32 = mybir.dt.float32
    dth = mybir.dt.float16
    nc = bacc.Bacc(num_devices=NCORES)

    wst = nc.declare_dram_parameter("wst", [128, 256], dth, isOutput=False)
    xts = [
        nc.declare_dram_parameter(f"xt{s}", [XROWS, (S + 1) * F], dth, isOutput=False)
        for s in range(NSTREAM)
    ]
    outs = [
        nc.declare_dram_parameter(f"out{s}", [G, L * F], dt32, isOutput=True)
        for s in range(NSTREAM)
    ]

    with tile.TileContext(nc) as tc:
        with (
            tc.tile_pool(name="sb", bufs=1) as sb,
            tc.tile_pool(name="ps", bufs=3, space="PSUM") as ps,
        ):
            hblks = [
                sb.tile([110, (S + 1) * F], dth, tag=f"h{s}", name=f"h{s}")
                for s in range(NSTREAM)
            ]
            xtts = [
                sb.tile([XROWS, (S + 1) * F], dth, tag=f"x{s}", name=f"x{s}")
                for s in range(NSTREAM)
            ]
            wst_t = sb.tile([128, 256], dth)
            out_sbs = [
                sb.tile([32, L * F], dt32, tag=f"o{s}", name=f"o{s}")
                for s in range(NSTREAM)
            ]

            for s in range(NSTREAM):
                nc.vector.memset(hblks[s][:, 0:F], 0.0)

            # sync: [slab0-s0, wst, s0's later slabs]; scalar: s1 slabs.
            # (Many 3-queue variants measured slower; consumers' DMA
            # waits are coarse whole-queue bundles and the first LDW
            # lands ~2us after wst's transfer due to completion-receipt
            # latency regardless of arrangement.)
            blocks = [0, 2, 8, 16, S + 1]
            nc.sync.dma_start(xtts[0][:, : 2 * F], xts[0][:, : 2 * F])
            nc.sync.dma_start(wst_t[:], wst[:])
            for bi, (lo, hi) in enumerate(zip(blocks[:-1], blocks[1:])):
                fl, fh = lo * F, hi * F
                if bi > 0:
                    nc.sync.dma_start(xtts[0][:, fl:fh], xts[0][:, fl:fh])
                nc.scalar.dma_start(xtts[1][:, fl:fh], xts[1][:, fl:fh])

            out_vs = [o[:].rearrange("p (l f) -> p l f", f=F) for o in out_sbs]

            for u in range(S + 1):
                pres = []
                for s in range(NSTREAM):
                    pre = ps.tile([M, F], dt32, tag=f"pre{s}", name=f"pre{s}_{u}")
                    nc.tensor.matmul(
                        pre[:], wst_t[0:XROWS, 128 : 256],
                        xtts[s][:, u * F : (u + 1) * F],
                        start=True, stop=False,
                    )
                    nc.tensor.matmul(
                        pre[:], wst_t[0:110, 0:128],
                        hblks[s][:, u * F : (u + 1) * F],
                        start=False, stop=True,
                    )
                    pres.append(pre)
                if u < S:
                    for s in range(NSTREAM):
                        nc.scalar.activation(
                            hblks[s][:, (u + 1) * F : (u + 2) * F],
                            pres[s][0:110, :],
                            mybir.ActivationFunctionType.Tanh,
                        )
                if u >= B + 1:
                    for s in range(NSTREAM):
                        nc.vector.tensor_copy(
                            out_vs[s][:, u - (B + 1), :], pres[s][96:128, :]
                        )
                if u == B + (L // 2) + 1:
                    for s in range(NSTREAM):
                        nc.sync.dma_start(
                            outs[s][:, : (L // 2) * F],
                            out_sbs[s][14:25, : (L // 2) * F],
                        )
                if u == B + L - 2:
                    for s in range(NSTREAM):
                        nc.sync.dma_start(
                            outs[s][:, (L // 2) * F : (L - 3) * F],
                            out_sbs[s][14:25, (L // 2) * F : (L - 3) * F],
                        )
            # final slabs on separate queues, in parallel
            nc.scalar.dma_start(
                outs[0][:, (L - 3) * F :], out_sbs[0][14:25, (L - 3) * F :]
            )
            nc.sync.dma_start(
                outs[1][:, (L - 3) * F :], out_sbs[1][14:25, (L - 3) * F :]
            )

    nc.compile()
    return nc


def _prep_inputs(src, W_ih, W_hh, b_ih, b_hh, W_fc, b_fc):
    src = np.asarray(src, np.float32).reshape(T, IN)
    W_ih = np.asarray(W_ih, np.float32)
    W_hh = np.asarray(W_hh, np.float32)
    b_ih = np.asarray(b_ih, np.float32)
    b_hh = np.asarray(b_hh, np.float32)
    W_fc = np.asarray(W_fc, np.float32)
    b_fc = np.asarray(b_fc, np.float32)
    bias = b_ih + b_hh

    # stationary tile [128, 256] fp16:
    #  cols 0..120   : chain stationary (lhsT rows 0..109 = h rows)
    #  cols 128..248 : xp stationary (lhsT rows 0..65 = src + ones rows)
    wst = np.zeros((128, 256), np.float32)
    for g in range(G):
        for j in range(HID):
            col = 10 * g + j
            wst[10 * g : 10 * g + 10, col] = W_hh[j, :]       # chain: h rows
            wst[10 * g + j, 110 + g] = W_fc[0, j]             # chain: out cols
            wst[5 * g : 5 * g + 5, 128 + col] = W_ih[j, :]    # xp: src rows
            wst[55 + g, 128 + col] = bias[j]                  # xp: bias
        wst[55 + g, 128 + 110 + g] = b_fc[0]                  # xp: b_fc
    wst16 = wst.astype(np.float16)

    # per-(core, stream) xt arrays [XROWS, (S+1)*F] fp16
    src16 = np.zeros((T + 1, IN), np.float16)
    src16[:T] = src.astype(np.float16)

    g_i = np.arange(G)[:, None, None]
    u_i = np.arange(S + 1)[None, :, None]
    c_i = np.arange(F)[None, None, :]
    rel = (g_i * F + c_i) * L + (u_i - B)  # relative timestep, (G, S+1, F)

    xt_list = []
    for k in range(NCORES):
        per_stream = []
        for s in range(NSTREAM):
            base = k * TC + s * SEG
            t_glob = base + rel
            valid = (t_glob >= 0) & (t_glob < T)
            idx = np.where(valid, t_glob, T)  # T -> zero row of src16
            x = src16[idx]                                  # (G, S+1, F, IN)
            x = np.transpose(x, (0, 3, 1, 2))               # (G, IN, S+1, F)
            # ones row: bias needed when this step is real (valid), b_fc
            # needed when the PREVIOUS step was real (its output is
            # harvested this step) -> OR of valid with its u-shift.
            vprev = np.zeros_like(valid)
            vprev[:, 1:, :] = valid[:, :-1, :]
            ones_row = (valid | vprev).astype(np.float16)
            arr = np.empty((XROWS, (S + 1) * F), np.float16)
            arr[: 5 * G] = x.reshape(5 * G, (S + 1) * F)
            arr[5 * G :] = ones_row.reshape(G, (S + 1) * F)
            per_stream.append(arr)
        xt_list.append(per_stream)
    return wst16, xt_list


def kernel(src, W_ih, W_hh, b_ih, b_hh, W_fc, b_fc):
    from concourse.bass_utils import run_bass_kernel_spmd

    if "nc" not in _COMPILED:
        _COMPILED["nc"] = _build_kernel()
    nc = _COMPILED["nc"]

    wst16, xt_list = _prep_inputs(src, W_ih, W_hh, b_ih, b_hh, W_fc, b_fc)
    in_maps = []
    for k in range(NCORES):
        m = {"wst": wst16}
        for s in range(NSTREAM):
            m[f"xt{s}"] = xt_list[k][s]
        in_maps.append(m)
    res = run_bass_kernel_spmd(nc, in_maps, list(range(NCORES)))

    full_out = np.empty(T, np.float32)
    for k in range(NCORES):
        for s in range(NSTREAM):
            arr = res.results[k][f"out{s}"].reshape(G, L, F)
            arr = arr.transpose(0, 2, 1).reshape(CS * L)  # (g, c, t) flat
            base = k * TC + s * SEG
            full_out[base : base + SEG] = arr[:SEG]
    return full_out.reshape(T, 1, OUT).astype(np.float32)


# revision 5
# speedup vs baseline: 1.0213x; 1.0213x over previous
"""Trainium2 Bass kernel for a small Elman RNN over a very long sequence.

Model (matches the torch/jax reference):
    xp_t  = W_ih @ x_t + b_ih + b_hh
    h_t   = tanh(xp_t + W_hh @ h_{t-1}),  h_{-1} = 0
    out_t = W_fc @ h_t + b_fc

Chunked-scan with burn-in (the recurrence is strongly contractive, so a
B-step replay of the preceding timesteps from h=0 converges to the true
state).  All matmul operands are fp16 (1 PE pass/row vs 4 for f32r;
fp16's 11-bit mantissa keeps quantization at the f32r-noise level).

Per-core layout (8 cores, each owns TC = 65536 contiguous steps),
NSTREAM=2 interleaved scan streams so one stream's matmuls overlap the
other stream's tanh on the serial chain:
  - G=11 chunk groups x F=176 chunk columns per stream, L=17 real steps
    per chunk, B=10 burn-in steps, S=B+L scan steps.
  - hblk (fp16, [110, (S+1)*F]): h state, group g at partitions
    10g..10g+9; block u holds the h that step u consumes.
  - xt (fp16, [66, (S+1)*F]): src features, group g at partitions
    5g..5g+4; rows 55+g = per-group validity row (1.0 inside the global
    timestep range) that carries bias+b_fc through the stationary --
    this makes the global first chunk's burn-in exact (h stays 0), no
    host fixup needed.
  - Two accumulating matmuls per step per stream into one PSUM tile
    [121, F]: xp = W_ih x + biases (start), then += W_hh h (stop).
    Stationary cols 110..120 hold W_fc, so the same pair also emits the
    PREVIOUS step's fc outputs (+b_fc via the validity rows) for free.
  - ACT tanh PSUM->SBUF writes block u+1 (rows 0..109, fp16).
  - DVE copies PSUM out rows [11, F] into a t-major out_sb tile so the
    output DMA is contiguous; the host untransposes (c, t) for free.
"""

import numpy as np

T = 524288
IN, HID, OUT = 5, 10, 1
NCORES = 8
TC = T // NCORES

G = 11             # chunk groups (10 partitions of h each)
F = 176            # chunk columns per group (matmul free dim)
NSTREAM = 2
B = 12             # burn-in steps
L = 17             # real steps per chunk
S = B + L          # scan steps (tanh per step); S+1 matmul iterations
CS = G * F         # chunks per stream
SEG = TC // NSTREAM
M = 128            # stationary cols: 110 pre + 11 out (+7 zero pad)
XROWS = 5 * G + G  # 55 src rows + 11 per-group ones rows

_COMPILED = {}


def _build_kernel():
    import concourse.bacc as bacc
    import concourse.mybir as mybir
    from concourse import tile

    dt32 = mybir.dt.float32
    dth = mybir.dt.float16
    nc = bacc.Bacc(num_devices=NCORES)

    wst = nc.declare_dram_parameter("wst", [128, 256], dth, isOutput=False)
    xts = [
        nc.declare_dram_parameter(f"xt{s}", [XROWS, (S + 1) * F], dth, isOutput=False)
        for s in range(NSTREAM)
    ]
    outs = [
        nc.declare_dram_parameter(f"out{s}", [G, L * F], dt32, isOutput=True)
        for s in range(NSTREAM)
    ]

    with tile.TileContext(nc) as tc:
        with (
            tc.tile_pool(name="sb", bufs=1) as sb,
            tc.tile_pool(name="ps", bufs=3, space="PSUM") as ps,
        ):
            hblks = [
                sb.tile([110, (S + 1) * F], dth, tag=f"h{s}", name=f"h{s}")
                for s in range(NSTREAM)
            ]
            xtts = [
                sb.tile([XROWS, (S + 1) * F], dth, tag=f"x{s}", name=f"x{s}")
                for s in range(NSTREAM)
            ]
            wst_t = sb.tile([128, 256], dth)
            out_sbs = [
                sb.tile([32, L * F], dt32, tag=f"o{s}", name=f"o{s}")
                for s in range(NSTREAM)
            ]

            for s in range(NSTREAM):
                nc.vector.memset(hblks[s][:, 0:F], 0.0)

            # sync: [slab0-s0, wst, s0's later slabs]; scalar: s1 slabs.
            # (Many 3-queue variants measured slower; consumers' DMA
            # waits are coarse whole-queue bundles and the first LDW
            # lands ~2us after wst's transfer due to completion-receipt
            # latency regardless of arrangement.)
            blocks = [0, 3, 9, 17, S + 1]
            nc.sync.dma_start(xtts[0][:, : 3 * F], xts[0][:, : 3 * F])
            nc.sync.dma_start(wst_t[:], wst[:])
            for bi, (lo, hi) in enumerate(zip(blocks[:-1], blocks[1:])):
                fl, fh = lo * F, hi * F
                if bi > 0:
                    nc.sync.dma_start(xtts[0][:, fl:fh], xts[0][:, fl:fh])
                nc.scalar.dma_start(xtts[1][:, fl:fh], xts[1][:, fl:fh])

            out_vs = [o[:].rearrange("p (l f) -> p l f", f=F) for o in out_sbs]

            for u in range(S + 1):
                pres = []
                for s in range(NSTREAM):
                    pre = ps.tile([M, F], dt32, tag=f"pre{s}", name=f"pre{s}_{u}")
                    nc.tensor.matmul(
                        pre[:], wst_t[0:XROWS, 128 : 256],
                        xtts[s][:, u * F : (u + 1) * F],
                        start=True, stop=False,
                    )
                    nc.tensor.matmul(
                        pre[:], wst_t[0:110, 0:128],
                        hblks[s][:, u * F : (u + 1) * F],
                        start=False, stop=True,
                    )
                    pres.append(pre)
                if u < S:
                    for s in range(NSTREAM):
                        nc.scalar.activation(
                            hblks[s][:, (u + 1) * F : (u + 2) * F],
                            pres[s][0:110, :],
                            mybir.ActivationFunctionType.Tanh,
                        )
                if u >= B + 1:
                    for s in range(NSTREAM):
                        nc.vector.tensor_copy(
                            out_vs[s][:, u - (B + 1), :], pres[s][96:128, :]
                        )
                if u == B + (L // 2) + 1:
                    for s in range(NSTREAM):
                        nc.sync.dma_start(
                            outs[s][:, : (L // 2) * F],
                            out_sbs[s][14:25, : (L // 2) * F],
                        )
                if u == B + L - 2:
                    for s in range(NSTREAM):
                        nc.sync.dma_start(
                            outs[s][:, (L // 2) * F : (L - 3) * F],
                            out_sbs[s][14:25, (L // 2) * F : (L - 3) * F],
                        )
            # final slabs on separate queues, in parallel
            nc.scalar.dma_start(
                outs[0][:, (L - 3) * F :], out_sbs[0][14:25, (L - 3) * F :]
            )
            nc.sync.dma_start(
                outs[1][:, (L - 3) * F :], out_sbs[1][14:25, (L - 3) * F :]
            )

    nc.compile()
    return nc


def _prep_inputs(src, W_ih, W_hh, b_ih, b_hh, W_fc, b_fc):
    src = np.asarray(src, np.float32).reshape(T, IN)
    W_ih = np.asarray(W_ih, np.float32)
    W_hh = np.asarray(W_hh, np.float32)
    b_ih = np.asarray(b_ih, np.float32)
    b_hh = np.asarray(b_hh, np.float32)
    W_fc = np.asarray(W_fc, np.float32)
    b_fc = np.asarray(b_fc, np.float32)
    bias = b_ih + b_hh

    # stationary tile [128, 256] fp16:
    #  cols 0..120   : chain stationary (lhsT rows 0..109 = h rows)
    #  cols 128..248 : xp stationary (lhsT rows 0..65 = src + ones rows)
    wst = np.zeros((128, 256), np.float32)
    for g in range(G):
        for j in range(HID):
            col = 10 * g + j
            wst[10 * g : 10 * g + 10, col] = W_hh[j, :]       # chain: h rows
            wst[10 * g + j, 110 + g] = W_fc[0, j]             # chain: out cols
            wst[5 * g : 5 * g + 5, 128 + col] = W_ih[j, :]    # xp: src rows
            wst[55 + g, 128 + col] = bias[j]                  # xp: bias
        wst[55 + g, 128 + 110 + g] = b_fc[0]                  # xp: b_fc
    wst16 = wst.astype(np.float16)

    # per-(core, stream) xt arrays [XROWS, (S+1)*F] fp16
    src16 = np.zeros((T + 1, IN), np.float16)
    src16[:T] = src.astype(np.float16)

    g_i = np.arange(G)[:, None, None]
    u_i = np.arange(S + 1)[None, :, None]
    c_i = np.arange(F)[None, None, :]
    rel = (g_i * F + c_i) * L + (u_i - B)  # relative timestep, (G, S+1, F)

    xt_list = []
    for k in range(NCORES):
        per_stream = []
        for s in range(NSTREAM):
            base = k * TC + s * SEG
            t_glob = base + rel
            valid = (t_glob >= 0) & (t_glob < T)
            idx = np.where(valid, t_glob, T)  # T -> zero row of src16
            x = src16[idx]                                  # (G, S+1, F, IN)
            x = np.transpose(x, (0, 3, 1, 2))               # (G, IN, S+1, F)
            # ones row: bias needed when this step is real (valid), b_fc
            # needed when the PREVIOUS step was real (its output is
            # harvested this step) -> OR of valid with its u-shift.
            vprev = np.zeros_like(valid)
            vprev[:, 1:, :] = valid[:, :-1, :]
            ones_row = (valid | vprev).astype(np.float16)
            arr = np.empty((XROWS, (S + 1) * F), np.float16)
            arr[: 5 * G] = x.reshape(5 * G, (S + 1) * F)
            arr[5 * G :] = ones_row.reshape(G, (S + 1) * F)
            per_stream.append(arr)
        xt_list.append(per_stream)
    return wst16, xt_list


def kernel(src, W_ih, W_hh, b_ih, b_hh, W_fc, b_fc):
    from concourse.bass_utils import run_bass_kernel_spmd

    if "nc" not in _COMPILED:
        _COMPILED["nc"] = _build_kernel()
    nc = _COMPILED["nc"]

    wst16, xt_list = _prep_inputs(src, W_ih, W_hh, b_ih, b_hh, W_fc, b_fc)
    in_maps = []
    for k in range(NCORES):
        m = {"wst": wst16}
        for s in range(NSTREAM):
            m[f"xt{s}"] = xt_list[k][s]
        in_maps.append(m)
    res = run_bass_kernel_spmd(nc, in_maps, list(range(NCORES)))

    full_out = np.empty(T, np.float32)
    for k in range(NCORES):
        for s in range(NSTREAM):
            arr = res.results[k][f"out{s}"].reshape(G, L, F)
            arr = arr.transpose(0, 2, 1).reshape(CS * L)  # (g, c, t) flat
            base = k * TC + s * SEG
            full_out[base : base + SEG] = arr[:SEG]
    return full_out.reshape(T, 1, OUT).astype(np.float32)
